# revision 1
# baseline (speedup 1.0000x reference)
"""Trainium2 Bass kernel for nn_NeuralSurface (8-layer MLP SDF with harmonic
embedding + skip concat), data-parallel over 8 NeuronCores.

Layout strategy: activations kept transposed in SBUF ([features, points]),
weights stationary fp16, PE matmuls K/M-chunked to 128. Harmonic sin/cos via
ScalarE Sin LUT after DVE range reduction to [-pi, pi] (magic-number
round-to-nearest). ReLU+bias split between ScalarE (activation Relu w/ bias)
and VectorE (tensor_scalar add+max) reading PSUM. n-tiles processed in pairs
so the PE always has independent matmul work while ReLUs complete.
"""

import numpy as np

import concourse.bacc as bacc
import concourse.mybir as mybir
import concourse.tile as tile
from concourse.bass_utils import run_bass_kernel_spmd

AF = mybir.ActivationFunctionType
ALU = mybir.AluOpType
F32 = mybir.dt.float32
F16 = mybir.dt.float16

N_CORES = 8
N = 262144
NPC = N // N_CORES  # 32768 points per core
NT = 512  # points per n-tile (PSUM bank / fp32 moving-operand limit)
PAIRS = NPC // (2 * NT)  # 32
H = 256
E = 39
NHARM = 6
TWO_PI = float(2.0 * np.pi)
MAGIC = float(1.5 * 2.0**23)  # round-to-nearest via (x + M) - M

# ReLU engine split: half 0 -> ACT, half 1 -> DVE (even split; each PSUM
# pair drains through two engines in parallel).
DVE_RELU = {(li, 1): True for li in range(8)}

_CACHED = {}


def _build():
    nc = bacc.Bacc("TRN2")

    rep6 = nc.dram_tensor("rep6", [128, NPC], F32, kind="ExternalInput").ap()
    ptsh = nc.dram_tensor("ptsh", [3, NPC], F16, kind="ExternalInput").ap()
    w0h = nc.dram_tensor("w0h", [128, H], F16, kind="ExternalInput").ap()
    wkh = {
        i: nc.dram_tensor(f"w{i}h", [H, H], F16, kind="ExternalInput").ap()
        for i in (1, 2, 3, 5, 6, 7)
    }
    w4eh = nc.dram_tensor("w4eh", [128, H], F16, kind="ExternalInput").ap()
    w4ah = nc.dram_tensor("w4ah", [128, H], F16, kind="ExternalInput").ap()
    w4bh = nc.dram_tensor("w4bh", [128, H], F16, kind="ExternalInput").ap()
    wsdfh = nc.dram_tensor("wsdfh", [H, 1], F16, kind="ExternalInput").ap()
    bmat = nc.dram_tensor("bmat", [128, 16], F32, kind="ExternalInput").ap()
    bsdf1 = nc.dram_tensor("bsdf1", [128, 1], F32, kind="ExternalInput").ap()
    # 2-D output (1-D ExternalOutput tensors fail NEFF load under bass2jax)
    out_o = nc.dram_tensor("out_o", [NPC // NT, NT], F32, kind="ExternalOutput").ap()

    with tile.TileContext(nc) as tc:
        with (
            tc.tile_pool(name="wp", bufs=1) as wp,
            tc.tile_pool(name="ep", bufs=4) as ep,
            tc.tile_pool(name="hp", bufs=4) as hp,
            tc.tile_pool(name="op", bufs=4) as op_,
            tc.tile_pool(name="pp", bufs=6, space="PSUM") as pp,
            tc.tile_pool(name="pf", bufs=1, space="PSUM") as pf,
        ):
            # ---- one-time weight / const loads ----
            w0s = wp.tile_from(w0h, name="w0s")  # [39, 256]
            wks = {
                i: (
                    wp.tile_from(wkh[i][0:128, :], name=f"wks{i}a"),
                    wp.tile_from(wkh[i][128:256, :], name=f"wks{i}b"),
                )
                for i in (1, 2, 3, 5, 6, 7)
            }
            w4es = wp.tile_from(w4eh, name="w4es")  # [128, 256] K-padded
            w4as = wp.tile_from(w4ah, name="w4as")  # [128, 256]
            w4bs = wp.tile_from(w4bh, name="w4bs")
            wsdf_a = wp.tile_from(wsdfh[0:128, :], name="wsdf_a")  # [128, 1]
            wsdf_b = wp.tile_from(wsdfh[128:256, :], name="wsdf_b")
            bms = wp.tile_from(bmat, name="bms")  # [128, 16]
            bsdfs = wp.tile_from(bsdf1, name="bsdfs")  # [1, 1]
            zcol = wp.tile([128, 1], F32, name="zcol")
            nc.vector.memset(zcol, 0.0)

            def wchunk(i, k, m):
                # lhsT [128, 128] slice: layer i, K-chunk k, M-half m
                return wks[i][k][:, bass_ts(m, 128)]

            for p in range(PAIRS):
                s = p * 2 * NT  # start point index of the pair (A at s, B at s+NT)
                W = 2 * NT  # pair-wide free size

                # ---- embedding (pair-wide, [128, 1024] ops) ----
                # rep6 rows carry t0 = x*2^j/(2pi) + phase (host-precomputed
                # exact scaling); rows 36:128 are zero -> Sin gives 0, so emb
                # is K-padded to 128 for free (full-K weight loads on PE).
                t0 = ep.tile([128, W], F32, tag="t0")
                nc.sync.dma_start(out=t0, in_=rep6[:, s:s + W])
                rr = ep.tile([128, W], F32, tag="rr")
                nc.vector.tensor_scalar(rr, t0, MAGIC, MAGIC, op0=ALU.add, op1=ALU.subtract)
                ys = ep.tile([128, W], F32, tag="ys")
                nc.vector.tensor_tensor(out=ys, in0=t0, in1=rr, op=ALU.subtract)

                emb = ep.tile([128, W], F16, tag="emb")
                nc.scalar.activation(emb, ys, AF.Sin, bias=zcol, scale=TWO_PI)
                nc.sync.dma_start(out=emb[36:39, :], in_=ptsh[:, s:s + W])

                # ---- MLP layers ----
                # h tile layout: [128, 4*NT]: A-half0, A-half1, B-half0, B-half1
                h_prev = None
                h3 = None
                for li in range(8):
                    h = hp.tile([128, 4 * NT], F16, tag="h")
                    # chunks: list of (weight tile [128,256], rhs per half_x)
                    if li == 0:
                        chunks = [(w0s, lambda hx: emb[:, bass_ts(hx, NT)])]
                    elif li == 4:
                        chunks = [
                            (w4es, lambda hx: emb[:, bass_ts(hx, NT)]),
                            (w4as, lambda hx, hp3=h3: hp3[:, bass_ts(2 * hx, NT)]),
                            (w4bs, lambda hx, hp3=h3: hp3[:, bass_ts(2 * hx + 1, NT)]),
                        ]
                    else:
                        chunks = [
                            (wks[li][0], lambda hx, hp_=h_prev: hp_[:, bass_ts(2 * hx, NT)]),
                            (wks[li][1], lambda hx, hp_=h_prev: hp_[:, bass_ts(2 * hx + 1, NT)]),
                        ]
                    ps = {(hx, m): pp.tile([128, NT], F32, tag="mm", name="psmm")
                          for hx in range(2) for m in range(2)}
                    last = len(chunks) - 1
                    for hx in range(2):
                        for m in range(2):
                            for ci, (wt, rhs) in enumerate(chunks):
                                nc.tensor.matmul(
                                    ps[(hx, m)], wt[:, bass_ts(m, 128)], rhs(hx),
                                    start=(ci == 0), stop=(ci == last),
                                )
                    # ReLU + bias -> h
                    for half_x in range(2):
                        for m in range(2):
                            dst = h[:, bass_ts(2 * half_x + m, NT)]
                            bias_ap = bms[:, li * 2 + m:li * 2 + m + 1]
                            if DVE_RELU.get((li, m), False):
                                nc.vector.tensor_scalar(
                                    dst, ps[(half_x, m)], bias_ap, 0.0,
                                    op0=ALU.add, op1=ALU.max,
                                )
                            else:
                                nc.scalar.activation(
                                    dst, ps[(half_x, m)], AF.Relu, bias=bias_ap,
                                )
                    if li == 3:
                        h3 = h
                    h_prev = h

                # ---- final SDF layer (M=1), col-group packed: A at array
                # col 0, B at array col 32 -> the two tiles' matmuls overlap
                # on the PE. Separate PSUM banks (same-bank dual accumulation
                # groups + DVE read crashed the exec unit).
                psfa = pf.tile([1, NT], F32, tag="finA")
                psfb_t = pf.tile([33, NT], F32, tag="finB")
                psfb = psfb_t[32:33, :]
                nc.tensor.matmul(
                    psfa, wsdf_a, h_prev[:, bass_ts(0, NT)],
                    start=True, stop=False, tile_position=(0, 0),
                    skip_group_check=True,
                )
                nc.tensor.matmul(
                    psfb, wsdf_a, h_prev[:, bass_ts(2, NT)],
                    start=True, stop=False, tile_position=(0, 32),
                    skip_group_check=True,
                )
                nc.tensor.matmul(
                    psfa, wsdf_b, h_prev[:, bass_ts(1, NT)],
                    start=False, stop=True, tile_position=(0, 0),
                    skip_group_check=True,
                )
                nc.tensor.matmul(
                    psfb, wsdf_b, h_prev[:, bass_ts(3, NT)],
                    start=False, stop=True, tile_position=(0, 32),
                    skip_group_check=True,
                )
                oa = op_.tile([1, NT], F32, tag="oa")
                nc.scalar.activation(oa, psfa, AF.Identity, bias=bsdfs[0:1, 0:1])
                ob = op_.tile([1, NT], F32, tag="ob")
                nc.scalar.activation(ob, psfb, AF.Identity, bias=bsdfs[0:1, 0:1])
                nc.sync.dma_start(out=out_o[2 * p:2 * p + 1, :], in_=oa)
                nc.sync.dma_start(out=out_o[2 * p + 1:2 * p + 2, :], in_=ob)
    nc.compile()
    return nc


def bass_ts(i, size):
    return slice(i * size, (i + 1) * size)


def _prep_maps(points, ws, bs, wsdf, bsdf):
    pts = np.ascontiguousarray(points, dtype=np.float32).reshape(N, 3)
    freqs = (2.0 ** np.arange(NHARM)).astype(np.float32)
    fcol18 = (np.repeat(freqs[None, :], 3, axis=0).reshape(18, 1) / TWO_PI).astype(
        np.float32
    )

    bmat = np.zeros((128, 16), dtype=np.float32)
    for i in range(8):
        for m in range(2):
            bmat[:, i * 2 + m] = bs[i][m * 128:(m + 1) * 128]

    w0p = np.zeros((128, H), dtype=np.float16)
    w0p[0:E, :] = ws[0].astype(np.float16)
    w4ep = np.zeros((128, H), dtype=np.float16)
    w4ep[0:E, :] = ws[4][0:E, :].astype(np.float16)
    common = {
        "w0h": w0p,
        "w4eh": w4ep,
        "w4ah": ws[4][E:E + 128, :].astype(np.float16),
        "w4bh": ws[4][E + 128:E + 256, :].astype(np.float16),
        "wsdfh": wsdf.astype(np.float16),
        "bmat": bmat,
        "bsdf1": np.full((128, 1), float(np.ravel(bsdf)[0]), dtype=np.float32),
    }
    for i in (1, 2, 3, 5, 6, 7):
        common[f"w{i}h"] = ws[i].astype(np.float16)

    in_maps = []
    for c in range(N_CORES):
        sl = pts[c * NPC:(c + 1) * NPC]  # [NPC, 3]
        ptsT = np.ascontiguousarray(sl.T)  # [3, NPC]
        rep3 = np.repeat(ptsT, NHARM, axis=0)  # [18, NPC]
        t18 = rep3 * fcol18  # x * 2^j / (2pi), exact fp32 scaling
        rep6 = np.zeros((128, NPC), dtype=np.float32)
        rep6[0:18], rep6[18:36] = t18, t18 + np.float32(0.25)
        m = dict(common)
        m["rep6"] = rep6
        m["ptsh"] = ptsT.astype(np.float16)
        in_maps.append(m)
    return in_maps


def kernel(
    points, w0, b0, w1, b1, w2, b2, w3, b3, w4, b4, w5, b5, w6, b6, w7, b7,
    wsdf, bsdf,
):
    ws = [np.asarray(w, dtype=np.float32) for w in (w0, w1, w2, w3, w4, w5, w6, w7)]
    bs = [np.asarray(b, dtype=np.float32) for b in (b0, b1, b2, b3, b4, b5, b6, b7)]
    in_maps = _prep_maps(
        np.asarray(points), ws, bs,
        np.asarray(wsdf, dtype=np.float32), np.asarray(bsdf, dtype=np.float32),
    )

    if "nc" not in _CACHED:
        _CACHED["nc"] = _build()
    nc = _CACHED["nc"]

    res = run_bass_kernel_spmd(nc, in_maps, core_ids=list(range(N_CORES)))
    out = np.concatenate(
        [res.results[c]["out_o"] for c in range(N_CORES)], axis=0
    ).reshape(N, 1).astype(np.float32)
    return out



# revision 2
# speedup vs baseline: 1.0146x; 1.0146x over previous
"""Trainium2 Bass kernel for nn_NeuralSurface (8-layer MLP SDF with harmonic
embedding + skip concat), data-parallel over 8 NeuronCores.

v2 layout strategy:
- Activations transposed in SBUF ([features, points]); weights stationary fp16;
  PE matmuls K/M-chunked to 128, N-tile NT=512 (one PSUM bank).
- K=39 embedding matmuls (layer 0 + layer 4's emb chunk) row-packed: even tile
  in array rows 0-63, odd tile in rows 64-127 via tile_position, running
  concurrently -> half the PE slots, and the host-side embedding args pack two
  tiles per column block (halves DMA + Sin/round work).
- Layer 0 of pair p+1 is software-pipelined into pair p (emitted after l4), so
  a pair starts at l1 with h0 already drained -> no pair-boundary PE bubble.
- Tile-outer MM order per layer ([A: m0c0,m0c1,m1c0,m1c1][B: ...]) gives every
  ReLU drain >=5 matmul-slots of cover before its consumer.
- Harmonic sin: host precomputes turn-domain args; GpSimd does the magic-number
  round-to-nearest range reduction; ScalarE Sin LUT produces the embedding.
- ReLU drains split ACT(17)/DVE(15) per pair; SDF finals on DVE.
"""

import numpy as np

import concourse.bacc as bacc
import concourse.mybir as mybir
import concourse.tile as tile
from concourse.bass_utils import run_bass_kernel_spmd

AF = mybir.ActivationFunctionType
ALU = mybir.AluOpType
F32 = mybir.dt.float32
F16 = mybir.dt.float16

N_CORES = 8
N = 262144
NPC = N // N_CORES  # 32768 points per core
NT = 512  # points per n-tile (PSUM bank limit for fp32)
PAIRS = NPC // (2 * NT)  # 32
H = 256
E = 39
NHARM = 6
TWO_PI = float(2.0 * np.pi)
MAGIC = float(1.5 * 2.0**23)  # round-to-nearest via (x + M) - M

# Drain engine split: ACT takes all m=0 drains + (4,0,1); DVE the rest.
ACT_DRAIN = {(li, t, 0) for li in range(8) for t in (0, 1)} | {(4, 0, 1)}

_CACHED = {}


def bass_ts(i, size):
    return slice(i * size, (i + 1) * size)


def _build():
    nc = bacc.Bacc("TRN2")

    rep6p = nc.dram_tensor("rep6p", [128, NPC // 2], F32, kind="ExternalInput").ap()
    ptseh = nc.dram_tensor("ptseh", [3, NPC // 2], F16, kind="ExternalInput").ap()
    ptsoh = nc.dram_tensor("ptsoh", [3, NPC // 2], F16, kind="ExternalInput").ap()
    w0f = nc.dram_tensor("w0f", [128, H], F16, kind="ExternalInput").ap()
    wkh = {
        i: nc.dram_tensor(f"w{i}h", [H, H], F16, kind="ExternalInput").ap()
        for i in (1, 2, 3, 5, 6, 7)
    }
    w4ef = nc.dram_tensor("w4ef", [128, H], F16, kind="ExternalInput").ap()
    w4ah = nc.dram_tensor("w4ah", [128, H], F16, kind="ExternalInput").ap()
    w4bh = nc.dram_tensor("w4bh", [128, H], F16, kind="ExternalInput").ap()
    wsdfh = nc.dram_tensor("wsdfh", [H, 1], F16, kind="ExternalInput").ap()
    bmat = nc.dram_tensor("bmat", [128, 16], F32, kind="ExternalInput").ap()
    bsdf1 = nc.dram_tensor("bsdf1", [128, 1], F32, kind="ExternalInput").ap()
    out_o = nc.dram_tensor("out_o", [NPC // NT, NT], F32, kind="ExternalOutput").ap()

    with tile.TileContext(nc) as tc:
        with (
            tc.tile_pool(name="wp", bufs=1) as wp,
            tc.tile_pool(name="ep", bufs=3) as ep,
            tc.tile_pool(name="embp", bufs=3) as embp,
            tc.tile_pool(name="hp", bufs=5) as hp,
            tc.tile_pool(name="op", bufs=3) as op_,
            tc.tile_pool(name="pp", bufs=6, space="PSUM") as pp,
            tc.tile_pool(name="pf", bufs=1, space="PSUM") as pf,
        ):
            # ---- one-time weight / const loads ----
            w0s = wp.tile_from(w0f, name="w0s")  # rows 0:39 / 64:103 = w0
            wks = {
                i: (
                    wp.tile_from(wkh[i][0:128, :], name=f"wks{i}a"),
                    wp.tile_from(wkh[i][128:256, :], name=f"wks{i}b"),
                )
                for i in (1, 2, 3, 5, 6, 7)
            }
            w4es = wp.tile_from(w4ef, name="w4es")
            w4as = wp.tile_from(w4ah, name="w4as")
            w4bs = wp.tile_from(w4bh, name="w4bs")
            wsdf_a = wp.tile_from(wsdfh[0:128, :], name="wsdf_a")
            wsdf_b = wp.tile_from(wsdfh[128:256, :], name="wsdf_b")
            bms = wp.tile_from(bmat, name="bms")  # [128, 16]
            bsdfs = wp.tile_from(bsdf1, name="bsdfs")
            zcol = wp.tile([128, 1], F32, name="zcol")
            nc.vector.memset(zcol, 0.0)

            def drain(li, t, m, ps, h):
                dst = h[:, bass_ts(2 * t + m, NT)]
                bias_ap = bms[:, li * 2 + m:li * 2 + m + 1]
                if (li, t, m) in ACT_DRAIN:
                    nc.scalar.activation(dst, ps, AF.Relu, bias=bias_ap)
                else:
                    nc.vector.tensor_scalar(
                        dst, ps, bias_ap, 0.0, op0=ALU.add, op1=ALU.max
                    )

            def emit_emb(p):
                # embedding for pair p: even tile rows 0:39, odd tile 64:103
                t0 = ep.tile([128, NT], F32, tag="t0")
                nc.sync.dma_start(out=t0, in_=rep6p[:, bass_ts(p, NT)])
                rr = ep.tile([128, NT], F32, tag="rr")
                nc.gpsimd.tensor_scalar(
                    rr, t0, MAGIC, MAGIC, op0=ALU.add, op1=ALU.subtract
                )
                ys = ep.tile([128, NT], F32, tag="ys")
                nc.gpsimd.tensor_tensor(out=ys, in0=t0, in1=rr, op=ALU.subtract)
                emb = embp.tile([128, NT], F16, tag="emb")
                nc.scalar.activation(emb, ys, AF.Sin, bias=zcol, scale=TWO_PI)
                nc.sync.dma_start(out=emb[36:39, :], in_=ptseh[:, bass_ts(p, NT)])
                nc.sync.dma_start(out=emb[100:103, :], in_=ptsoh[:, bass_ts(p, NT)])
                return emb

            def emit_l0(emb):
                # layer 0, row-packed: tile A (cols of even tile) in array rows
                # 0-63, tile B in rows 64-127; the two MMs per m run
                # concurrently on the PE.
                h = hp.tile([128, 4 * NT], F16, tag="h")
                ps = {
                    (t, m): pp.tile([128, NT], F32, tag="ps", name="psmm")
                    for t in (0, 1) for m in (0, 1)
                }
                for m in (0, 1):
                    nc.tensor.matmul(
                        ps[(0, m)], w0s[0:64, bass_ts(m, 128)], emb[0:64, :],
                        start=True, stop=True, tile_position=(0, 0),
                        skip_group_check=True,
                    )
                    nc.tensor.matmul(
                        ps[(1, m)], w0s[64:128, bass_ts(m, 128)], emb[64:128, :],
                        start=True, stop=True, tile_position=(64, 0),
                        skip_group_check=True,
                    )
                for t in (0, 1):
                    for m in (0, 1):
                        drain(0, t, m, ps[(t, m)], h)
                return h

            def emit_layer(li, h_prev):
                # layers 1,2,3,5,6,7: K=256 in 2 chunks, tile-outer order
                h = hp.tile([128, 4 * NT], F16, tag="h")
                ps = {
                    (t, m): pp.tile([128, NT], F32, tag="ps", name="psmm")
                    for t in (0, 1) for m in (0, 1)
                }
                for t in (0, 1):
                    for m in (0, 1):
                        for ci in (0, 1):
                            nc.tensor.matmul(
                                ps[(t, m)], wks[li][ci][:, bass_ts(m, 128)],
                                h_prev[:, bass_ts(2 * t + ci, NT)],
                                start=(ci == 0), stop=(ci == 1),
                            )
                        drain(li, t, m, ps[(t, m)], h)
                return h

            def emit_l4(emb, h3):
                # layer 4: K = 39(emb, row-packed) + 256(h3, 2 full chunks)
                h = hp.tile([128, 4 * NT], F16, tag="h")
                ps = {
                    (t, m): pp.tile([128, NT], F32, tag="ps", name="psmm")
                    for t in (0, 1) for m in (0, 1)
                }
                for m in (0, 1):
                    nc.tensor.matmul(
                        ps[(0, m)], w4es[0:64, bass_ts(m, 128)], emb[0:64, :],
                        start=True, stop=False, tile_position=(0, 0),
                        skip_group_check=True,
                    )
                    nc.tensor.matmul(
                        ps[(1, m)], w4es[64:128, bass_ts(m, 128)], emb[64:128, :],
                        start=True, stop=False, tile_position=(64, 0),
                        skip_group_check=True,
                    )
                for t in (0, 1):
                    for m in (0, 1):
                        for ci, wt in ((0, w4as), (1, w4bs)):
                            nc.tensor.matmul(
                                ps[(t, m)], wt[:, bass_ts(m, 128)],
                                h3[:, bass_ts(2 * t + ci, NT)],
                                start=False, stop=(ci == 1),
                                skip_group_check=True,
                            )
                        drain(4, t, m, ps[(t, m)], h)
                return h

            def emit_sdf(p, h7):
                # final SDF layer (M=1): A at array col group 0, B at col
                # group 1; separate PSUM banks.
                psfa = pf.tile([1, NT], F32, tag="finA")
                psfb_t = pf.tile([33, NT], F32, tag="finB")
                psfb = psfb_t[32:33, :]
                nc.tensor.matmul(
                    psfa, wsdf_a, h7[:, bass_ts(0, NT)],
                    start=True, stop=False, tile_position=(0, 0),
                    skip_group_check=True,
                )
                nc.tensor.matmul(
                    psfa, wsdf_b, h7[:, bass_ts(1, NT)],
                    start=False, stop=True, tile_position=(0, 0),
                    skip_group_check=True,
                )
                nc.tensor.matmul(
                    psfb, wsdf_a, h7[:, bass_ts(2, NT)],
                    start=True, stop=False, tile_position=(0, 32),
                    skip_group_check=True,
                )
                nc.tensor.matmul(
                    psfb, wsdf_b, h7[:, bass_ts(3, NT)],
                    start=False, stop=True, tile_position=(0, 32),
                    skip_group_check=True,
                )
                oa = op_.tile([1, NT], F32, tag="oa")
                nc.vector.tensor_scalar(
                    oa, psfa, bsdfs[0:1, 0:1], 0.0, op0=ALU.add, op1=ALU.add
                )
                ob = op_.tile([1, NT], F32, tag="ob")
                nc.vector.tensor_scalar(
                    ob, psfb, bsdfs[0:1, 0:1], 0.0, op0=ALU.add, op1=ALU.add
                )
                nc.sync.dma_start(out=out_o[2 * p:2 * p + 1, :], in_=oa)
                nc.sync.dma_start(out=out_o[2 * p + 1:2 * p + 2, :], in_=ob)

            # ---- main pipeline ----
            emb_cur = emit_emb(0)
            h0_cur = emit_l0(emb_cur)
            for p in range(PAIRS):
                emb_next = emit_emb(p + 1) if p + 1 < PAIRS else None
                h1 = emit_layer(1, h0_cur)
                h2 = emit_layer(2, h1)
                h3 = emit_layer(3, h2)
                h4 = emit_l4(emb_cur, h3)
                if emb_next is not None:
                    h0_next = emit_l0(emb_next)
                h5 = emit_layer(5, h4)
                h6 = emit_layer(6, h5)
                h7 = emit_layer(7, h6)
                emit_sdf(p, h7)
                if emb_next is not None:
                    emb_cur, h0_cur = emb_next, h0_next
    nc.compile()
    return nc


def _prep_maps(points, ws, bs, wsdf, bsdf):
    pts = np.ascontiguousarray(points, dtype=np.float32).reshape(N, 3)
    freqs = (2.0 ** np.arange(NHARM)).astype(np.float32)
    fcol18 = (np.repeat(freqs[None, :], 3, axis=0).reshape(18, 1) / TWO_PI).astype(
        np.float32
    )

    bmat = np.zeros((128, 16), dtype=np.float32)
    for i in range(8):
        for m in range(2):
            bmat[:, i * 2 + m] = bs[i][m * 128:(m + 1) * 128]

    w0f = np.zeros((128, H), dtype=np.float16)
    w0f[0:E, :] = ws[0].astype(np.float16)
    w0f[64:64 + E, :] = ws[0].astype(np.float16)
    w4ef = np.zeros((128, H), dtype=np.float16)
    w4ef[0:E, :] = ws[4][0:E, :].astype(np.float16)
    w4ef[64:64 + E, :] = ws[4][0:E, :].astype(np.float16)
    common = {
        "w0f": w0f,
        "w4ef": w4ef,
        "w4ah": ws[4][E:E + 128, :].astype(np.float16),
        "w4bh": ws[4][E + 128:E + 256, :].astype(np.float16),
        "wsdfh": wsdf.astype(np.float16),
        "bmat": bmat,
        "bsdf1": np.full((128, 1), float(np.ravel(bsdf)[0]), dtype=np.float32),
    }
    for i in (1, 2, 3, 5, 6, 7):
        common[f"w{i}h"] = ws[i].astype(np.float16)

    in_maps = []
    for c in range(N_CORES):
        sl = pts[c * NPC:(c + 1) * NPC]  # [NPC, 3]
        ptsT = np.ascontiguousarray(sl.T)  # [3, NPC]
        rep3 = np.repeat(ptsT, NHARM, axis=0)  # [18, NPC]
        t18 = rep3 * fcol18  # x * 2^j / (2pi), exact fp32 scaling
        # pack: even tile -> rows 0:36, odd tile -> rows 64:100
        t18v = t18.reshape(18, PAIRS, 2, NT)
        ev = t18v[:, :, 0, :].reshape(18, NPC // 2)
        od = t18v[:, :, 1, :].reshape(18, NPC // 2)
        rep6p = np.zeros((128, NPC // 2), dtype=np.float32)
        rep6p[0:18], rep6p[18:36] = ev, ev + np.float32(0.25)
        rep6p[64:82], rep6p[82:100] = od, od + np.float32(0.25)
        ptsv = ptsT.reshape(3, PAIRS, 2, NT)
        m = dict(common)
        m["rep6p"] = rep6p
        m["ptseh"] = np.ascontiguousarray(
            ptsv[:, :, 0, :].reshape(3, NPC // 2)
        ).astype(np.float16)
        m["ptsoh"] = np.ascontiguousarray(
            ptsv[:, :, 1, :].reshape(3, NPC // 2)
        ).astype(np.float16)
        in_maps.append(m)
    return in_maps


def kernel(
    points, w0, b0, w1, b1, w2, b2, w3, b3, w4, b4, w5, b5, w6, b6, w7, b7,
    wsdf, bsdf,
):
    ws = [np.asarray(w, dtype=np.float32) for w in (w0, w1, w2, w3, w4, w5, w6, w7)]
    bs = [np.asarray(b, dtype=np.float32) for b in (b0, b1, b2, b3, b4, b5, b6, b7)]
    in_maps = _prep_maps(
        np.asarray(points), ws, bs,
        np.asarray(wsdf, dtype=np.float32), np.asarray(bsdf, dtype=np.float32),
    )

    if "nc" not in _CACHED:
        _CACHED["nc"] = _build()
    nc = _CACHED["nc"]

    res = run_bass_kernel_spmd(nc, in_maps, core_ids=list(range(N_CORES)))
    out = np.concatenate(
        [res.results[c]["out_o"] for c in range(N_CORES)], axis=0
    ).reshape(N, 1).astype(np.float32)
    return out


# revision 4
# speedup vs baseline: 1.0920x; 1.0763x over previous
"""Trainium2 Bass kernel for nn_NeuralSurface (8-layer MLP SDF with harmonic
embedding + skip concat), data-parallel over 8 NeuronCores.

v3 layout strategy:
- Activations transposed in SBUF ([features, points]); weights stationary fp16;
  PE matmuls K/M-chunked to 128, N-tile NT=512 (one PSUM bank).
- K=39 embedding matmuls (layer 0 + layer 4's emb chunk) row-packed: even tile
  in array rows 0-63, odd tile in rows 64-127 via tile_position, running
  concurrently -> half the PE slots; host-side embedding args pack two tiles
  per column block (halves DMA + Sin work).
- Layer 0 of pair p+1 is software-pipelined into pair p (emitted after l4), so
  a pair starts at l1 with h0 already drained -> no pair-boundary PE bubble.
- Tile-outer MM order per layer ([A: m0c0,m0c1,m1c0,m1c1][B: ...]) gives every
  ReLU drain >=5 matmul-slots of cover before its consumer.
- Harmonic sin: host does the range reduction (ships ys = t - round(t) in fp16,
  packed two tiles per column block); on-chip it is one DMA + one ScalarE Sin.
- All weights ship in one DRAM tensor (one DMA) to avoid serialized
  DMA-issue latency at startup.
- ReLU drains alternate ACT/DVE by (t+m) parity; SDF finals split 1/1.
"""

import numpy as np

import concourse.bacc as bacc
import concourse.mybir as mybir
import concourse.tile as tile
from concourse.bass_utils import run_bass_kernel_spmd

AF = mybir.ActivationFunctionType
ALU = mybir.AluOpType
F32 = mybir.dt.float32
F16 = mybir.dt.float16

N_CORES = 8
N = 262144
NPC = N // N_CORES  # 32768 points per core
NT = 512  # points per n-tile (PSUM bank limit for fp32)
PAIRS = NPC // (2 * NT)  # 32
H = 256
E = 39
NHARM = 6
TWO_PI = float(2.0 * np.pi)

# Weight columns inside the single packed weight tensor [128, WCOLS]:
# w0f, w4ef, then (a,b) chunks for layers 1,2,3,5,6,7, then w4a, w4b,
# then wsdf as two fp16 columns.
_WOFF = {}
_off = 0
for _name in ("w0f", "w4ef", "w1a", "w1b", "w2a", "w2b", "w3a", "w3b",
              "w5a", "w5b", "w6a", "w6b", "w7a", "w7b", "w4a", "w4b"):
    _WOFF[_name] = _off
    _off += H
_WOFF["wsdfa"] = _off
_WOFF["wsdfb"] = _off + 1
WCOLS = _off + 2

_CACHED = {}


def bass_ts(i, size):
    return slice(i * size, (i + 1) * size)


def _build():
    nc = bacc.Bacc("TRN2")

    ysh = nc.dram_tensor("ysh", [128, NPC // 2], F16, kind="ExternalInput").ap()
    ptseh = nc.dram_tensor("ptseh", [3, NPC // 2], F16, kind="ExternalInput").ap()
    ptsoh = nc.dram_tensor("ptsoh", [3, NPC // 2], F16, kind="ExternalInput").ap()
    wbh = nc.dram_tensor("wbh", [128, WCOLS], F16, kind="ExternalInput").ap()
    bm17h = nc.dram_tensor("bm17h", [128, 17], F32, kind="ExternalInput").ap()
    out_o = nc.dram_tensor("out_o", [NPC // NT, NT], F32, kind="ExternalOutput").ap()

    with tile.TileContext(nc) as tc:
        with (
            tc.tile_pool(name="wp", bufs=1) as wp,
            tc.tile_pool(name="ep", bufs=3) as ep,
            tc.tile_pool(name="embp", bufs=3) as embp,
            tc.tile_pool(name="hp", bufs=5) as hp,
            tc.tile_pool(name="op", bufs=3) as op_,
            tc.tile_pool(name="pp", bufs=6, space="PSUM") as pp,
            tc.tile_pool(name="pf", bufs=1, space="PSUM") as pf,
        ):
            zcol = wp.tile([128, 1], F32, name="zcol")
            nc.vector.memset(zcol, 0.0)

            def emit_emb(p):
                # embedding pair p: even tile rows 0:39, odd tile rows 64:103;
                # ys already range-reduced on host, Sin arg = 2*pi*ys.
                ys = ep.tile([128, NT], F16, tag="ys")
                nc.sync.dma_start(out=ys, in_=ysh[:, bass_ts(p, NT)])
                emb = embp.tile([128, NT], F16, tag="emb")
                nc.scalar.activation(emb, ys, AF.Sin, bias=zcol, scale=TWO_PI)
                nc.sync.dma_start(out=emb[36:39, :], in_=ptseh[:, bass_ts(p, NT)])
                nc.sync.dma_start(out=emb[100:103, :], in_=ptsoh[:, bass_ts(p, NT)])
                return emb

            # emb(0) DMAs first so they are not stuck behind the weight DMA.
            emb_cur = emit_emb(0)

            # ---- one-time weight / const loads (single DMA each) ----
            wb = wp.tile_from(wbh, name="wb")  # [128, WCOLS] fp16
            bms = wp.tile_from(bm17h, name="bms")  # [128, 17] fp32

            def wcol(name, m=None):
                off = _WOFF[name]
                if m is None:
                    return wb[:, off:off + 1]
                return wb[:, off + m * 128:off + (m + 1) * 128]

            def drain(li, t, m, ps, h):
                dst = h[:, bass_ts(2 * t + m, NT)]
                bias_ap = bms[:, li * 2 + m:li * 2 + m + 1]
                if (t + m) % 2 == 0:
                    nc.scalar.activation(dst, ps, AF.Relu, bias=bias_ap)
                else:
                    nc.vector.tensor_scalar(
                        dst, ps, bias_ap, 0.0, op0=ALU.add, op1=ALU.max
                    )

            def emit_l0(emb):
                # layer 0, row-packed: even tile in array rows 0-63, odd tile
                # in rows 64-127; the two MMs per m run concurrently.
                h = hp.tile([128, 4 * NT], F16, tag="h")
                ps = {
                    (t, m): pp.tile([128, NT], F32, tag="ps", name="psmm")
                    for t in (0, 1) for m in (0, 1)
                }
                for m in (0, 1):
                    nc.tensor.matmul(
                        ps[(0, m)], wcol("w0f", m)[0:64, :], emb[0:64, :],
                        start=True, stop=True, tile_position=(0, 0),
                        skip_group_check=True,
                    )
                    nc.tensor.matmul(
                        ps[(1, m)], wcol("w0f", m)[64:128, :], emb[64:128, :],
                        start=True, stop=True, tile_position=(64, 0),
                        skip_group_check=True,
                    )
                for t in (0, 1):
                    for m in (0, 1):
                        drain(0, t, m, ps[(t, m)], h)
                return h

            def emit_layer(li, h_prev):
                # layers 1,2,3,5,6,7: K=256 in 2 chunks, tile-outer order
                h = hp.tile([128, 4 * NT], F16, tag="h")
                ps = {
                    (t, m): pp.tile([128, NT], F32, tag="ps", name="psmm")
                    for t in (0, 1) for m in (0, 1)
                }
                for t in (0, 1):
                    for m in (0, 1):
                        for ci in (0, 1):
                            nc.tensor.matmul(
                                ps[(t, m)], wcol(f"w{li}{'ab'[ci]}", m),
                                h_prev[:, bass_ts(2 * t + ci, NT)],
                                start=(ci == 0), stop=(ci == 1),
                            )
                        drain(li, t, m, ps[(t, m)], h)
                return h

            def emit_l4(emb, h3):
                # layer 4: K = 39(emb, row-packed) + 256(h3, 2 full chunks)
                h = hp.tile([128, 4 * NT], F16, tag="h")
                ps = {
                    (t, m): pp.tile([128, NT], F32, tag="ps", name="psmm")
                    for t in (0, 1) for m in (0, 1)
                }
                for m in (0, 1):
                    nc.tensor.matmul(
                        ps[(0, m)], wcol("w4ef", m)[0:64, :], emb[0:64, :],
                        start=True, stop=False, tile_position=(0, 0),
                        skip_group_check=True,
                    )
                    nc.tensor.matmul(
                        ps[(1, m)], wcol("w4ef", m)[64:128, :], emb[64:128, :],
                        start=True, stop=False, tile_position=(64, 0),
                        skip_group_check=True,
                    )
                for t in (0, 1):
                    for m in (0, 1):
                        for ci, wname in ((0, "w4a"), (1, "w4b")):
                            nc.tensor.matmul(
                                ps[(t, m)], wcol(wname, m),
                                h3[:, bass_ts(2 * t + ci, NT)],
                                start=False, stop=(ci == 1),
                                skip_group_check=True,
                            )
                        drain(4, t, m, ps[(t, m)], h)
                return h

            def emit_sdf(p, h7):
                # final SDF layer (M=1): A at array col group 0, B at col
                # group 1; separate PSUM banks.
                psfa = pf.tile([1, NT], F32, tag="finA")
                psfb_t = pf.tile([33, NT], F32, tag="finB")
                psfb = psfb_t[32:33, :]
                nc.tensor.matmul(
                    psfa, wcol("wsdfa"), h7[:, bass_ts(0, NT)],
                    start=True, stop=False, tile_position=(0, 0),
                    skip_group_check=True,
                )
                nc.tensor.matmul(
                    psfa, wcol("wsdfb"), h7[:, bass_ts(1, NT)],
                    start=False, stop=True, tile_position=(0, 0),
                    skip_group_check=True,
                )
                nc.tensor.matmul(
                    psfb, wcol("wsdfa"), h7[:, bass_ts(2, NT)],
                    start=True, stop=False, tile_position=(0, 32),
                    skip_group_check=True,
                )
                nc.tensor.matmul(
                    psfb, wcol("wsdfb"), h7[:, bass_ts(3, NT)],
                    start=False, stop=True, tile_position=(0, 32),
                    skip_group_check=True,
                )
                bsdf_ap = bms[0:1, 16:17]
                oa = op_.tile([1, NT], F32, tag="oa")
                nc.scalar.activation(oa, psfa, AF.Identity, bias=bsdf_ap)
                ob = op_.tile([1, NT], F32, tag="ob")
                nc.vector.tensor_scalar(
                    ob, psfb, bsdf_ap, 0.0, op0=ALU.add, op1=ALU.add
                )
                nc.sync.dma_start(out=out_o[2 * p:2 * p + 1, :], in_=oa)
                nc.sync.dma_start(out=out_o[2 * p + 1:2 * p + 2, :], in_=ob)

            # ---- main pipeline ----
            h0_cur = emit_l0(emb_cur)
            for p in range(PAIRS):
                emb_next = emit_emb(p + 1) if p + 1 < PAIRS else None
                h1 = emit_layer(1, h0_cur)
                h2 = emit_layer(2, h1)
                h3 = emit_layer(3, h2)
                h4 = emit_l4(emb_cur, h3)
                if emb_next is not None:
                    h0_next = emit_l0(emb_next)
                h5 = emit_layer(5, h4)
                h6 = emit_layer(6, h5)
                h7 = emit_layer(7, h6)
                emit_sdf(p, h7)
                if emb_next is not None:
                    emb_cur, h0_cur = emb_next, h0_next
    nc.compile()
    return nc


def _prep_maps(points, ws, bs, wsdf, bsdf):
    pts = np.ascontiguousarray(points, dtype=np.float32).reshape(N, 3)
    freqs = (2.0 ** np.arange(NHARM)).astype(np.float32)
    fcol18 = (np.repeat(freqs[None, :], 3, axis=0).reshape(18, 1) / TWO_PI).astype(
        np.float32
    )

    bm17 = np.zeros((128, 17), dtype=np.float32)
    for i in range(8):
        for m in range(2):
            bm17[:, i * 2 + m] = bs[i][m * 128:(m + 1) * 128]
    bm17[:, 16] = float(np.ravel(bsdf)[0])

    wb = np.zeros((128, WCOLS), dtype=np.float16)
    wb[0:E, _WOFF["w0f"]:_WOFF["w0f"] + H] = ws[0].astype(np.float16)
    wb[64:64 + E, _WOFF["w0f"]:_WOFF["w0f"] + H] = ws[0].astype(np.float16)
    wb[0:E, _WOFF["w4ef"]:_WOFF["w4ef"] + H] = ws[4][0:E].astype(np.float16)
    wb[64:64 + E, _WOFF["w4ef"]:_WOFF["w4ef"] + H] = ws[4][0:E].astype(np.float16)
    for i in (1, 2, 3, 5, 6, 7):
        wb[:, _WOFF[f"w{i}a"]:_WOFF[f"w{i}a"] + H] = ws[i][0:128].astype(np.float16)
        wb[:, _WOFF[f"w{i}b"]:_WOFF[f"w{i}b"] + H] = ws[i][128:256].astype(np.float16)
    wb[:, _WOFF["w4a"]:_WOFF["w4a"] + H] = ws[4][E:E + 128].astype(np.float16)
    wb[:, _WOFF["w4b"]:_WOFF["w4b"] + H] = ws[4][E + 128:E + 256].astype(np.float16)
    wb[:, _WOFF["wsdfa"]] = wsdf[0:128, 0].astype(np.float16)
    wb[:, _WOFF["wsdfb"]] = wsdf[128:256, 0].astype(np.float16)

    common = {"wbh": wb, "bm17h": bm17}

    in_maps = []
    for c in range(N_CORES):
        sl = pts[c * NPC:(c + 1) * NPC]  # [NPC, 3]
        ptsT = np.ascontiguousarray(sl.T)  # [3, NPC]
        rep3 = np.repeat(ptsT, NHARM, axis=0)  # [18, NPC]
        t18 = rep3 * fcol18  # x * 2^j / (2pi), exact fp32 scaling
        t18c = t18 + np.float32(0.25)  # cos rows as shifted sin
        ys18 = (t18 - np.rint(t18)).astype(np.float32)
        ys18c = (t18c - np.rint(t18c)).astype(np.float32)
        # pack: even tile -> rows 0:36, odd tile -> rows 64:100
        ev_s = ys18.reshape(18, PAIRS, 2, NT)[:, :, 0, :].reshape(18, NPC // 2)
        od_s = ys18.reshape(18, PAIRS, 2, NT)[:, :, 1, :].reshape(18, NPC // 2)
        ev_c = ys18c.reshape(18, PAIRS, 2, NT)[:, :, 0, :].reshape(18, NPC // 2)
        od_c = ys18c.reshape(18, PAIRS, 2, NT)[:, :, 1, :].reshape(18, NPC // 2)
        ysh = np.zeros((128, NPC // 2), dtype=np.float16)
        ysh[0:18], ysh[18:36] = ev_s, ev_c
        ysh[64:82], ysh[82:100] = od_s, od_c
        ptsv = ptsT.reshape(3, PAIRS, 2, NT)
        m = dict(common)
        m["ysh"] = ysh
        m["ptseh"] = np.ascontiguousarray(
            ptsv[:, :, 0, :].reshape(3, NPC // 2)
        ).astype(np.float16)
        m["ptsoh"] = np.ascontiguousarray(
            ptsv[:, :, 1, :].reshape(3, NPC // 2)
        ).astype(np.float16)
        in_maps.append(m)
    return in_maps


def kernel(
    points, w0, b0, w1, b1, w2, b2, w3, b3, w4, b4, w5, b5, w6, b6, w7, b7,
    wsdf, bsdf,
):
    ws = [np.asarray(w, dtype=np.float32) for w in (w0, w1, w2, w3, w4, w5, w6, w7)]
    bs = [np.asarray(b, dtype=np.float32) for b in (b0, b1, b2, b3, b4, b5, b6, b7)]
    in_maps = _prep_maps(
        np.asarray(points), ws, bs,
        np.asarray(wsdf, dtype=np.float32), np.asarray(bsdf, dtype=np.float32),
    )

    if "nc" not in _CACHED:
        _CACHED["nc"] = _build()
    nc = _CACHED["nc"]

    res = run_bass_kernel_spmd(nc, in_maps, core_ids=list(range(N_CORES)))
    out = np.concatenate(
        [res.results[c]["out_o"] for c in range(N_CORES)], axis=0
    ).reshape(N, 1).astype(np.float32)
    return out


# revision 10
# speedup vs baseline: 1.1291x; 1.0339x over previous
"""Trainium2 Bass kernel for nn_NeuralSurface (8-layer MLP SDF with harmonic
embedding + skip concat), data-parallel over 8 NeuronCores.

v3 layout strategy:
- Activations transposed in SBUF ([features, points]); weights stationary fp16;
  PE matmuls K/M-chunked to 128, N-tile NT=512 (one PSUM bank).
- K=39 embedding matmuls (layer 0 + layer 4's emb chunk) row-packed: even tile
  in array rows 0-63, odd tile in rows 64-127 via tile_position, running
  concurrently -> half the PE slots; host-side embedding args pack two tiles
  per column block (halves DMA + Sin work).
- Layer 0 of pair p+1 is software-pipelined into pair p (emitted after l4), so
  a pair starts at l1 with h0 already drained -> no pair-boundary PE bubble.
- Tile-outer MM order per layer ([A: m0c0,m0c1,m1c0,m1c1][B: ...]) gives every
  ReLU drain >=5 matmul-slots of cover before its consumer.
- Harmonic sin: host does the range reduction (ships ys = t - round(t) in fp16,
  packed two tiles per column block); on-chip it is one DMA + one ScalarE Sin.
- All weights ship in one DRAM tensor (one DMA) to avoid serialized
  DMA-issue latency at startup.
- ReLU drains alternate ACT/DVE by (t+m) parity; SDF finals split 1/1.
"""

import numpy as np

import concourse.bacc as bacc
import concourse.mybir as mybir
import concourse.tile as tile
from concourse.bass_utils import run_bass_kernel_spmd

AF = mybir.ActivationFunctionType
ALU = mybir.AluOpType
F32 = mybir.dt.float32
F16 = mybir.dt.float16

N_CORES = 8
N = 262144
NPC = N // N_CORES  # 32768 points per core
NT = 512  # points per n-tile (PSUM bank limit for fp32)
PAIRS = NPC // (2 * NT)  # 32
H = 256
E = 39
NHARM = 6
TWO_PI = float(2.0 * np.pi)

# Weight columns inside the single packed weight tensor [128, WCOLS]:
# w0f, w4ef, then (a,b) chunks for layers 1,2,3,5,6,7, then w4a, w4b,
# then wsdf as two fp16 columns.
_WOFF = {}
_off = 0
for _name in ("w0f", "w4ef", "w1a", "w1b", "w2a", "w2b", "w3a", "w3b",
              "w5a", "w5b", "w6a", "w6b", "w7a", "w7b", "w4a", "w4b"):
    _WOFF[_name] = _off
    _off += H
# wsdf chunks padded to M=128 (col 0 = wsdf, rest zero) so the SDF matmuls
# keep the full-array config -> LDWEIGHTS stays pipelined (M=1 config
# switches cost ~94ns each side).
_WOFF["wsdfa"] = _off
_WOFF["wsdfb"] = _off + 128
WCOLS = _off + 256

_CACHED = {}


def bass_ts(i, size):
    return slice(i * size, (i + 1) * size)


def _build():
    nc = bacc.Bacc("TRN2")

    ysh = nc.dram_tensor("ysh", [128, NPC // 2], F16, kind="ExternalInput").ap()
    ptseh = nc.dram_tensor("ptseh", [3, NPC // 2], F16, kind="ExternalInput").ap()
    ptsoh = nc.dram_tensor("ptsoh", [3, NPC // 2], F16, kind="ExternalInput").ap()
    wbh = nc.dram_tensor("wbh", [128, WCOLS], F16, kind="ExternalInput").ap()
    bm17h = nc.dram_tensor("bm17h", [128, 17], F32, kind="ExternalInput").ap()
    out_o = nc.dram_tensor("out_o", [NPC // NT, NT], F32, kind="ExternalOutput").ap()

    with tile.TileContext(nc) as tc:
        with (
            tc.tile_pool(name="wp", bufs=1) as wp,
            tc.tile_pool(name="ep", bufs=3) as ep,
            tc.tile_pool(name="embp", bufs=3) as embp,
            tc.tile_pool(name="hp", bufs=6) as hp,
            tc.tile_pool(name="op", bufs=3) as op_,
            tc.tile_pool(name="pp", bufs=6, space="PSUM") as pp,
            tc.tile_pool(name="pf", bufs=1, space="PSUM") as pf,
        ):
            zcol = wp.tile([128, 1], F32, name="zcol")
            nc.vector.memset(zcol, 0.0)

            # HAM warmup: ~60 tiny matmuls on the zero column while the
            # input DMAs land, so the PE clock gate is at 8/8 when the real
            # matmul stream starts (~3.5us of PE activity needed).
            warm = pf.tile([1, NT], F32, tag="finA", name="warm")
            for _ in range(60):
                nc.tensor.matmul(
                    warm[0:1, 0:1], zcol, zcol,
                    start=True, stop=True, skip_group_check=True,
                )

            def emit_emb(p):
                # embedding pair p: even tile rows 0:39, odd tile rows 64:103;
                # ys already range-reduced on host, Sin arg = 2*pi*ys.
                ys = ep.tile([128, NT], F16, tag="ys")
                nc.sync.dma_start(out=ys, in_=ysh[:, bass_ts(p, NT)])
                emb = embp.tile([128, NT], F16, tag="emb")
                nc.scalar.activation(emb, ys, AF.Sin, bias=zcol, scale=TWO_PI)
                nc.sync.dma_start(out=emb[36:39, :], in_=ptseh[:, bass_ts(p, NT)])
                nc.sync.dma_start(out=emb[100:103, :], in_=ptsoh[:, bass_ts(p, NT)])
                return emb

            # emb(0) DMAs first so they are not stuck behind the weight DMA.
            emb_cur = emit_emb(0)

            # ---- one-time weight / const loads (single DMA each) ----
            wb = wp.tile_from(wbh, name="wb")  # [128, WCOLS] fp16
            bms = wp.tile_from(bm17h, name="bms")  # [128, 17] fp32

            def wcol(name, m=None):
                off = _WOFF[name]
                if m is None:
                    return wb[:, off:off + 1]
                return wb[:, off + m * 128:off + (m + 1) * 128]

            def drain(li, t, m, ps, h):
                dst = h[:, bass_ts(2 * t + m, NT)]
                bias_ap = bms[:, li * 2 + m:li * 2 + m + 1]
                if (t + m) % 2 == 0:
                    nc.scalar.activation(dst, ps, AF.Relu, bias=bias_ap)
                else:
                    nc.vector.tensor_scalar(
                        dst, ps, bias_ap, 0.0, op0=ALU.add, op1=ALU.max
                    )

            def emit_l0(emb):
                # layer 0, row-packed: even tile in array rows 0-63, odd tile
                # in rows 64-127; the two MMs per m run concurrently.
                h = hp.tile([128, 4 * NT], F16, tag="h")
                ps = {
                    (t, m): pp.tile([128, NT], F32, tag="ps", name="psmm")
                    for t in (0, 1) for m in (0, 1)
                }
                for m in (0, 1):
                    nc.tensor.matmul(
                        ps[(0, m)], wcol("w0f", m)[0:64, :], emb[0:64, :],
                        start=True, stop=True, tile_position=(0, 0),
                        skip_group_check=True,
                    )
                    nc.tensor.matmul(
                        ps[(1, m)], wcol("w0f", m)[64:128, :], emb[64:128, :],
                        start=True, stop=True, tile_position=(64, 0),
                        skip_group_check=True,
                    )
                for t in (0, 1):
                    for m in (0, 1):
                        drain(0, t, m, ps[(t, m)], h)
                return h

            def emit_layer(li, h_prev):
                # layers 1,2,3,5,6,7: K=256 in 2 chunks, tile-outer order
                h = hp.tile([128, 4 * NT], F16, tag="h")
                ps = {
                    (t, m): pp.tile([128, NT], F32, tag="ps", name="psmm")
                    for t in (0, 1) for m in (0, 1)
                }
                for t in (0, 1):
                    for m in (0, 1):
                        for ci in (0, 1):
                            nc.tensor.matmul(
                                ps[(t, m)], wcol(f"w{li}{'ab'[ci]}", m),
                                h_prev[:, bass_ts(2 * t + ci, NT)],
                                start=(ci == 0), stop=(ci == 1),
                            )
                        drain(li, t, m, ps[(t, m)], h)
                return h

            def emit_l4(emb, h3):
                # layer 4: K = 39(emb, row-packed) + 256(h3, 2 full chunks)
                h = hp.tile([128, 4 * NT], F16, tag="h")
                ps = {
                    (t, m): pp.tile([128, NT], F32, tag="ps", name="psmm")
                    for t in (0, 1) for m in (0, 1)
                }
                for m in (0, 1):
                    nc.tensor.matmul(
                        ps[(0, m)], wcol("w4ef", m)[0:64, :], emb[0:64, :],
                        start=True, stop=False, tile_position=(0, 0),
                        skip_group_check=True,
                    )
                    nc.tensor.matmul(
                        ps[(1, m)], wcol("w4ef", m)[64:128, :], emb[64:128, :],
                        start=True, stop=False, tile_position=(64, 0),
                        skip_group_check=True,
                    )
                for t in (0, 1):
                    for m in (0, 1):
                        for ci, wname in ((0, "w4a"), (1, "w4b")):
                            nc.tensor.matmul(
                                ps[(t, m)], wcol(wname, m),
                                h3[:, bass_ts(2 * t + ci, NT)],
                                start=False, stop=(ci == 1),
                                skip_group_check=True,
                            )
                        drain(4, t, m, ps[(t, m)], h)
                return h

            def emit_sdf(p, h7):
                # final SDF layer: wsdf padded to M=128 (row 0 is the real
                # output) so the array config matches the layer matmuls and
                # LDWEIGHTS stays pipelined.
                psfa = pf.tile([128, NT], F32, tag="finA")
                psfb = pf.tile([128, NT], F32, tag="finB")
                for psf, t in ((psfa, 0), (psfb, 1)):
                    nc.tensor.matmul(
                        psf, wcol("wsdfa", 0), h7[:, bass_ts(2 * t, NT)],
                        start=True, stop=False,
                    )
                    nc.tensor.matmul(
                        psf, wcol("wsdfb", 0), h7[:, bass_ts(2 * t + 1, NT)],
                        start=False, stop=True,
                    )
                bsdf_ap = bms[0:1, 16:17]
                oa = op_.tile([1, NT], F32, tag="oa")
                nc.scalar.activation(oa, psfa[0:1, :], AF.Identity, bias=bsdf_ap)
                ob = op_.tile([1, NT], F32, tag="ob")
                nc.vector.tensor_scalar(
                    ob, psfb[0:1, :], bsdf_ap, 0.0, op0=ALU.add, op1=ALU.add
                )
                nc.sync.dma_start(out=out_o[2 * p:2 * p + 1, :], in_=oa)
                nc.sync.dma_start(out=out_o[2 * p + 1:2 * p + 2, :], in_=ob)

            # ---- main pipeline ----
            h0_cur = emit_l0(emb_cur)
            for p in range(PAIRS):
                emb_next = emit_emb(p + 1) if p + 1 < PAIRS else None
                h1 = emit_layer(1, h0_cur)
                h2 = emit_layer(2, h1)
                # l0 of the next pair sits here so its PSUM banks recycle
                # quickly (l3/l4 reuse them) and l5 reuses l3's banks, two
                # layers back -> no slot-wait stalls.
                if emb_next is not None:
                    h0_next = emit_l0(emb_next)
                h3 = emit_layer(3, h2)
                h4 = emit_l4(emb_cur, h3)
                h5 = emit_layer(5, h4)
                h6 = emit_layer(6, h5)
                h7 = emit_layer(7, h6)
                emit_sdf(p, h7)
                if emb_next is not None:
                    emb_cur, h0_cur = emb_next, h0_next
    nc.compile()
    return nc


def _prep_maps(points, ws, bs, wsdf, bsdf):
    pts = np.ascontiguousarray(points, dtype=np.float32).reshape(N, 3)
    freqs = (2.0 ** np.arange(NHARM)).astype(np.float32)
    fcol18 = (np.repeat(freqs[None, :], 3, axis=0).reshape(18, 1) / TWO_PI).astype(
        np.float32
    )

    bm17 = np.zeros((128, 17), dtype=np.float32)
    for i in range(8):
        for m in range(2):
            bm17[:, i * 2 + m] = bs[i][m * 128:(m + 1) * 128]
    bm17[:, 16] = float(np.ravel(bsdf)[0])

    wb = np.zeros((128, WCOLS), dtype=np.float16)
    wb[0:E, _WOFF["w0f"]:_WOFF["w0f"] + H] = ws[0].astype(np.float16)
    wb[64:64 + E, _WOFF["w0f"]:_WOFF["w0f"] + H] = ws[0].astype(np.float16)
    wb[0:E, _WOFF["w4ef"]:_WOFF["w4ef"] + H] = ws[4][0:E].astype(np.float16)
    wb[64:64 + E, _WOFF["w4ef"]:_WOFF["w4ef"] + H] = ws[4][0:E].astype(np.float16)
    for i in (1, 2, 3, 5, 6, 7):
        wb[:, _WOFF[f"w{i}a"]:_WOFF[f"w{i}a"] + H] = ws[i][0:128].astype(np.float16)
        wb[:, _WOFF[f"w{i}b"]:_WOFF[f"w{i}b"] + H] = ws[i][128:256].astype(np.float16)
    wb[:, _WOFF["w4a"]:_WOFF["w4a"] + H] = ws[4][E:E + 128].astype(np.float16)
    wb[:, _WOFF["w4b"]:_WOFF["w4b"] + H] = ws[4][E + 128:E + 256].astype(np.float16)
    wb[:, _WOFF["wsdfa"]] = wsdf[0:128, 0].astype(np.float16)  # col 0; rest 0
    wb[:, _WOFF["wsdfb"]] = wsdf[128:256, 0].astype(np.float16)

    common = {"wbh": wb, "bm17h": bm17}

    in_maps = []
    for c in range(N_CORES):
        sl = pts[c * NPC:(c + 1) * NPC]  # [NPC, 3]
        ptsT = np.ascontiguousarray(sl.T)  # [3, NPC]
        rep3 = np.repeat(ptsT, NHARM, axis=0)  # [18, NPC]
        t18 = rep3 * fcol18  # x * 2^j / (2pi), exact fp32 scaling
        t18c = t18 + np.float32(0.25)  # cos rows as shifted sin
        ys18 = (t18 - np.rint(t18)).astype(np.float32)
        ys18c = (t18c - np.rint(t18c)).astype(np.float32)
        # pack: even tile -> rows 0:36, odd tile -> rows 64:100
        ev_s = ys18.reshape(18, PAIRS, 2, NT)[:, :, 0, :].reshape(18, NPC // 2)
        od_s = ys18.reshape(18, PAIRS, 2, NT)[:, :, 1, :].reshape(18, NPC // 2)
        ev_c = ys18c.reshape(18, PAIRS, 2, NT)[:, :, 0, :].reshape(18, NPC // 2)
        od_c = ys18c.reshape(18, PAIRS, 2, NT)[:, :, 1, :].reshape(18, NPC // 2)
        ysh = np.zeros((128, NPC // 2), dtype=np.float16)
        ysh[0:18], ysh[18:36] = ev_s, ev_c
        ysh[64:82], ysh[82:100] = od_s, od_c
        ptsv = ptsT.reshape(3, PAIRS, 2, NT)
        m = dict(common)
        m["ysh"] = ysh
        m["ptseh"] = np.ascontiguousarray(
            ptsv[:, :, 0, :].reshape(3, NPC // 2)
        ).astype(np.float16)
        m["ptsoh"] = np.ascontiguousarray(
            ptsv[:, :, 1, :].reshape(3, NPC // 2)
        ).astype(np.float16)
        in_maps.append(m)
    return in_maps


def kernel(
    points, w0, b0, w1, b1, w2, b2, w3, b3, w4, b4, w5, b5, w6, b6, w7, b7,
    wsdf, bsdf,
):
    ws = [np.asarray(w, dtype=np.float32) for w in (w0, w1, w2, w3, w4, w5, w6, w7)]
    bs = [np.asarray(b, dtype=np.float32) for b in (b0, b1, b2, b3, b4, b5, b6, b7)]
    in_maps = _prep_maps(
        np.asarray(points), ws, bs,
        np.asarray(wsdf, dtype=np.float32), np.asarray(bsdf, dtype=np.float32),
    )

    if "nc" not in _CACHED:
        _CACHED["nc"] = _build()
    nc = _CACHED["nc"]

    res = run_bass_kernel_spmd(nc, in_maps, core_ids=list(range(N_CORES)))
    out = np.concatenate(
        [res.results[c]["out_o"] for c in range(N_CORES)], axis=0
    ).reshape(N, 1).astype(np.float32)
    return out
